# revision 3
# baseline (speedup 1.0000x reference)
"""Trainium2 Bass kernel for AttentionSimple (linear/kernelized attention).

Computes, for x:[B,N,C], w_qkv:[C,3C], w_proj:[C,C], b_proj:[C]:
    qkv = x @ w_qkv -> split q,k,v per head (H=12, D=64)
    kv  = (k^T v) * D^-0.5          per (b, h)     [D, D]
    out = gelu(q) @ gelu(kv)        per (b, h)     [N, D]
    y   = out @ w_proj + b_proj

Sharding: data-parallel over batch B=16 across 8 NeuronCores (2 batches/core).
All matmuls run in bf16 with fp32 PSUM accumulation.

Dataflow per core (per batch b):
  pass 1: x tiles -> bf16 -> x^T via DMA-xbar transpose; k,v in natural
          [token, d] layout (lhsT = x^T chunk); per-head-pair k^T v
          accumulated in a persistent PSUM tile; q^T computed directly
          transposed (lhsT = w_q chunk, rhs = x^T) with gelu fused into the
          PSUM evacuation; gelu(kv * scale) into per-head [64,64] tiles.
  pass 2: attention o^T = gkv_h^T @ gq_h^T with two heads packed in
          disjoint 64-row/64-col PE sub-arrays (concurrent); projection
          consumes o^T directly (natural-layout output), bias folded in as
          a K=1 matmul row; contiguous DMA out.

Self-contained: hardcodes shapes; builds the Bass program, runs it SPMD on
cores 0-7 via bass_utils.run_bass_kernel_spmd, returns the gathered output.
"""

import numpy as np

import concourse.bacc as bacc
import concourse.bass as bass
import concourse.mybir as mybir
import concourse.tile as tile
from concourse.bass_utils import run_bass_kernel_spmd

F32 = mybir.dt.float32
BF16 = mybir.dt.bfloat16
GELU = mybir.ActivationFunctionType.Gelu
COPY = mybir.ActivationFunctionType.Copy
PSUM = bass.MemorySpace.PSUM

B, N, C = 16, 4096, 768
H, D = 12, 64
SCALE = D**-0.5
NCORES = 8
BPC = B // NCORES  # batches per core
CCH = C // 128  # 6 column chunks of 128
NTS = N // 512  # 8 slices of 512 tokens
NPAIR = H // 2  # 6 head pairs (128 cols each)


def _build_program():
    nc = bacc.Bacc("TRN2", target_bir_lowering=False, debug=False)

    x_d = nc.dram_tensor("x", [BPC, N, C], F32, kind="ExternalInput").ap()
    wq_d = nc.dram_tensor("w_qkv", [C, 3 * C], F32, kind="ExternalInput").ap()
    wp_d = nc.dram_tensor("w_proj", [C, C], F32, kind="ExternalInput").ap()
    bp_d = nc.dram_tensor("b_proj", [C], F32, kind="ExternalInput").ap()
    y_d = nc.dram_tensor("y", [BPC, N, C], F32, kind="ExternalOutput").ap()

    with tile.TileContext(nc) as tc:
        with (
            tc.tile_pool(name="weights", bufs=1) as wpool,
            tc.tile_pool(name="acts", bufs=1) as apool,
            tc.tile_pool(name="xin", bufs=3) as xpool,
            tc.tile_pool(name="xt", bufs=2) as xtpool,
            tc.tile_pool(name="kvnat", bufs=3) as kvpool,
            tc.tile_pool(name="ot", bufs=2) as otpool,
            tc.tile_pool(name="yout", bufs=3) as ypool,
            tc.tile_pool(name="ps_mm", bufs=3, space=PSUM) as ps_mm,
            tc.tile_pool(name="ps_q", bufs=2, space=PSUM) as ps_q,
            tc.tile_pool(name="ps_kv", bufs=1, space=PSUM) as ps_kv,
        ):
            # ---- weights / constants (once) ----
            w_qkv = wpool.tile([128, CCH, 3 * C], BF16)  # 27.6KB/part
            w_proj = wpool.tile([128, CCH, C], BF16)  # 9.2KB/part
            for cch in range(CCH):
                nc.gpsimd.dma_start(
                    w_qkv[:, cch, :], wq_d[cch * 128 : (cch + 1) * 128, :]
                )
                nc.gpsimd.dma_start(
                    w_proj[:, cch, :], wp_d[cch * 128 : (cch + 1) * 128, :]
                )
            b_row = wpool.tile([1, C], BF16)
            nc.gpsimd.dma_start(b_row[:], bp_d.unsqueeze(0))
            ones1 = wpool.tile([1, 128], BF16)
            nc.gpsimd.memset(ones1[:], 1.0)

            for b in range(BPC):
                # gqT: q^T with gelu applied, [c=768, t=4096] as 6 chunks
                gqT = apool.tile([128, CCH, N], BF16, tag="gqT")
                # gkv: per-pair [64,64] gelu(kv*scale); even head on
                # partitions 0:64, odd head on partitions 64:128
                gkv = apool.tile([128, NPAIR, 64], BF16, tag="gkv")

                # ================= pass 1: qkv + kv accumulation ============
                kv_acc = ps_kv.tile([128, NPAIR * 128], F32)
                for ts in range(NTS):
                    xT = xtpool.tile([128, CCH, 512], BF16)
                    for tc4 in range(4):
                        t0 = ts * 512 + tc4 * 128
                        x_bf = xpool.tile([128, C], BF16)
                        nc.gpsimd.dma_start(x_bf[:], x_d[b, t0 : t0 + 128, :])
                        for cch in range(CCH):
                            nc.sync.dma_start(
                                xT[:, cch, tc4 * 128 : tc4 * 128 + 128],
                                x_bf[:, cch * 128 : (cch + 1) * 128],
                                transpose=True,
                            )
                    # ---- k, v (natural layout) + kv pair accumulation ----
                    for tc4 in range(4):
                        tsl = slice(tc4 * 128, tc4 * 128 + 128)
                        k_nat = kvpool.tile([128, C], BF16, tag="k")
                        v_nat = kvpool.tile([128, C], BF16, tag="v")
                        for js in range(3):  # j in [768+512js, 768+512(js+1))
                            pmm = ps_mm.tile([128, 512], F32, tag="pmm")
                            for cch in range(CCH):
                                nc.tensor.matmul(
                                    pmm[:],
                                    xT[:, cch, tsl],
                                    w_qkv[:, cch, C + js * 512 : C + js * 512 + 512],
                                    start=(cch == 0),
                                    stop=(cch == CCH - 1),
                                )
                            # k cols: js0 -> k[0:512], js1 -> k[512:768]+v[0:256]
                            if js == 0:
                                nc.vector.tensor_copy(k_nat[:, 0:512], pmm[:])
                            elif js == 1:
                                nc.vector.tensor_copy(k_nat[:, 512:768], pmm[:, 0:256])
                                nc.vector.tensor_copy(v_nat[:, 0:256], pmm[:, 256:512])
                            else:
                                nc.vector.tensor_copy(v_nat[:, 256:768], pmm[:])
                        first = ts == 0 and tc4 == 0
                        last = ts == NTS - 1 and tc4 == 3
                        for pr in range(NPAIR):
                            psl = slice(pr * 128, pr * 128 + 128)
                            # start=True clears has_written for the WHOLE psum
                            # bank, so only the first matmul touching each bank
                            # may set it (bank0: pairs 0-3, bank1: pairs 4-5).
                            nc.tensor.matmul(
                                kv_acc[:, psl],
                                k_nat[:, psl],
                                v_nat[:, psl],
                                start=(first and pr in (0, 4)),
                                stop=last,
                                skip_group_check=True,
                            )
                    # ---- q^T chunks with fused gelu ----
                    for jch in range(CCH):
                        pq = ps_q.tile([128, 512], F32, tag="pq")
                        for cch in range(CCH):
                            nc.tensor.matmul(
                                pq[:],
                                w_qkv[:, cch, jch * 128 : (jch + 1) * 128],
                                xT[:, cch, 0:512],
                                start=(cch == 0),
                                stop=(cch == CCH - 1),
                            )
                        nc.scalar.activation(
                            gqT[:, jch, ts * 512 : ts * 512 + 512], pq[:], GELU
                        )
                # ---- gelu(kv * scale) into per-head tiles ----
                for pr in range(NPAIR):
                    c0 = pr * 128
                    nc.scalar.activation(
                        gkv[0:64, pr, :],
                        kv_acc[0:64, c0 : c0 + 64],
                        GELU,
                        scale=SCALE,
                    )
                    nc.scalar.activation(
                        gkv[64:128, pr, :],
                        kv_acc[64:128, c0 + 64 : c0 + 128],
                        GELU,
                        scale=SCALE,
                    )

                # ================= pass 2: attention + projection ===========
                for ts in range(NTS):
                    tq = slice(ts * 512, ts * 512 + 512)
                    oT = otpool.tile([128, NPAIR, 512], BF16)
                    for pr in range(NPAIR):
                        po = ps_mm.tile([128, 512], F32, tag="pmm")
                        # two heads in disjoint 64-row/64-col sub-arrays;
                        # the two matmuls execute concurrently on the PE.
                        nc.tensor.matmul(
                            po[0:64, :], gkv[0:64, pr, :], gqT[0:64, pr, tq]
                        )
                        nc.tensor.matmul(
                            po[64:128, :], gkv[64:128, pr, :], gqT[64:128, pr, tq]
                        )
                        nc.vector.tensor_copy(oT[:, pr, :], po[:])
                    for tc4 in range(4):
                        tsl = slice(tc4 * 128, tc4 * 128 + 128)
                        py0 = ps_mm.tile([128, 512], F32, tag="pmm")
                        py1 = ps_mm.tile([128, 256], F32, tag="pmm")
                        for cch in range(CCH):
                            nc.tensor.matmul(
                                py0[:],
                                oT[:, cch, tsl],
                                w_proj[:, cch, 0:512],
                                start=(cch == 0),
                                stop=False,
                                skip_group_check=True,
                            )
                            nc.tensor.matmul(
                                py1[:],
                                oT[:, cch, tsl],
                                w_proj[:, cch, 512:768],
                                start=(cch == 0),
                                stop=False,
                                skip_group_check=True,
                            )
                        nc.tensor.matmul(
                            py0[:], ones1[:], b_row[:, 0:512],
                            start=False, stop=True, skip_group_check=True,
                        )
                        nc.tensor.matmul(
                            py1[:], ones1[:], b_row[:, 512:768],
                            start=False, stop=True, skip_group_check=True,
                        )
                        y_sb = ypool.tile([128, C], F32)
                        nc.scalar.activation(y_sb[:, 0:512], py0[:], COPY)
                        nc.scalar.activation(y_sb[:, 512:768], py1[:], COPY)
                        t0 = ts * 512 + tc4 * 128
                        nc.sync.dma_start(y_d[b, t0 : t0 + 128, :], y_sb[:])

    nc.compile()
    return nc


_cached_nc = None


def kernel(x, w_qkv, w_proj, b_proj):
    global _cached_nc
    if _cached_nc is None:
        _cached_nc = _build_program()
    nc = _cached_nc

    x = np.ascontiguousarray(x, dtype=np.float32)
    in_maps = [
        {
            "x": x[i * BPC : (i + 1) * BPC],
            "w_qkv": np.asarray(w_qkv, dtype=np.float32),
            "w_proj": np.asarray(w_proj, dtype=np.float32),
            "b_proj": np.asarray(b_proj, dtype=np.float32),
        }
        for i in range(NCORES)
    ]
    res = run_bass_kernel_spmd(nc, in_maps, core_ids=list(range(NCORES)))
    out = np.concatenate([res.results[i]["y"] for i in range(NCORES)], axis=0)
    return out.astype(np.float32)


# revision 7
# speedup vs baseline: 1.2904x; 1.2904x over previous
"""Trainium2 Bass kernel for AttentionSimple (linear/kernelized attention).

Computes, for x:[B,N,C], w_qkv:[C,3C], w_proj:[C,C], b_proj:[C]:
    qkv = x @ w_qkv -> split q,k,v per head (H=12, D=64)
    kv  = (k^T v) * D^-0.5          per (b, h)     [D, D]
    out = gelu(q) @ gelu(kv)        per (b, h)     [N, D]
    y   = out @ w_proj + b_proj

Sharding: data-parallel over batch B=16 across 8 NeuronCores (2 batches/core).
All matmuls run in bf16 with fp32 PSUM accumulation.

Dataflow per core (per batch b):
  pass 1: x tiles -> bf16 -> x^T via DMA-xbar transpose; k,v in natural
          [token, d] layout (lhsT = x^T chunk); per-head-pair k^T v
          accumulated in a persistent PSUM tile; q^T computed directly
          transposed (lhsT = w_q chunk, rhs = x^T) with gelu fused into the
          PSUM evacuation; gelu(kv * scale) into per-head [64,64] tiles.
  pass 2: attention o^T = gkv_h^T @ gq_h^T with two heads packed in
          disjoint 64-row/64-col PE sub-arrays (concurrent); projection
          consumes o^T directly (natural-layout output), bias folded in as
          a K=1 matmul row; contiguous DMA out.

Self-contained: hardcodes shapes; builds the Bass program, runs it SPMD on
cores 0-7 via bass_utils.run_bass_kernel_spmd, returns the gathered output.
"""

import numpy as np

import concourse.bacc as bacc
import concourse.bass as bass
import concourse.mybir as mybir
import concourse.tile as tile
from concourse import masks
from concourse.bass_utils import run_bass_kernel_spmd

F32 = mybir.dt.float32
BF16 = mybir.dt.bfloat16
GELU = mybir.ActivationFunctionType.Gelu
COPY = mybir.ActivationFunctionType.Copy
PSUM = bass.MemorySpace.PSUM

B, N, C = 16, 4096, 768
H, D = 12, 64
SCALE = D**-0.5
NCORES = 8
BPC = B // NCORES  # batches per core
CCH = C // 128  # 6 column chunks of 128
NTS = N // 512  # 8 slices of 512 tokens
NPAIR = H // 2  # 6 head pairs (128 cols each)


def _build_program():
    nc = bacc.Bacc("TRN2", target_bir_lowering=False, debug=False)

    x_d = nc.dram_tensor("x", [BPC, N, C], F32, kind="ExternalInput").ap()
    wq_d = nc.dram_tensor("w_qkv", [C, 3 * C], F32, kind="ExternalInput").ap()
    wp_d = nc.dram_tensor("w_proj", [C, C], F32, kind="ExternalInput").ap()
    bp_d = nc.dram_tensor("b_proj", [C], F32, kind="ExternalInput").ap()
    y_d = nc.dram_tensor("y", [BPC, N, C], F32, kind="ExternalOutput").ap()

    with tile.TileContext(nc) as tc:
        with (
            tc.tile_pool(name="weights", bufs=1) as wpool,
            tc.tile_pool(name="acts", bufs=1) as apool,
            tc.tile_pool(name="xin", bufs=3) as xpool,
            tc.tile_pool(name="xt", bufs=2) as xtpool,
            tc.tile_pool(name="kvnat", bufs=3) as kvpool,
            tc.tile_pool(name="ot", bufs=2) as otpool,
            tc.tile_pool(name="yout", bufs=3) as ypool,
            tc.tile_pool(name="ps_mm", bufs=2, space=PSUM) as ps_mm,
            tc.tile_pool(name="ps_q", bufs=2, space=PSUM) as ps_q,
            tc.tile_pool(name="ps_kv", bufs=1, space=PSUM) as ps_kv,
            tc.tile_pool(name="ps_tr", bufs=2, space=PSUM) as ps_tr,
        ):
            # ---- weights / constants (once) ----
            w_qkv = wpool.tile([128, CCH, 3 * C], BF16)  # 27.6KB/part
            w_proj = wpool.tile([128, CCH, C], BF16)  # 9.2KB/part
            for cch in range(CCH):
                nc.gpsimd.dma_start(
                    w_qkv[:, cch, :], wq_d[cch * 128 : (cch + 1) * 128, :]
                )
                nc.gpsimd.dma_start(
                    w_proj[:, cch, :], wp_d[cch * 128 : (cch + 1) * 128, :]
                )
            b_row = wpool.tile([1, C], BF16)
            nc.gpsimd.dma_start(b_row[:], bp_d.unsqueeze(0))
            ones1 = wpool.tile([1, 128], BF16)
            nc.gpsimd.memset(ones1[:], 1.0)
            ident = wpool.tile([128, 128], BF16)
            masks.make_identity(nc, ident[:])

            for b in range(BPC):
                # gqT: q^T with gelu applied, [c=768, t=4096] as 6 chunks
                gqT = apool.tile([128, CCH, N], BF16, tag="gqT")
                # gkv: per-pair [64,64] gelu(kv*scale); even head on
                # partitions 0:64, odd head on partitions 64:128
                gkv = apool.tile([128, NPAIR, 64], BF16, tag="gkv")

                # ================= pass 1: qkv + kv accumulation ============
                kv_acc = ps_kv.tile([128, NPAIR * 128], F32)
                for ts in range(NTS):
                    xT = xtpool.tile([128, CCH, 512], BF16)
                    for tc4 in range(4):
                        t0 = ts * 512 + tc4 * 128
                        x_bf = xpool.tile([128, C], BF16)
                        nc.gpsimd.dma_start(x_bf[:], x_d[b, t0 : t0 + 128, :])
                        tr = ps_tr.tile([128, CCH * 128], BF16)
                        for cch in range(CCH):
                            nc.tensor.transpose(
                                tr[:, cch * 128 : (cch + 1) * 128],
                                x_bf[:, cch * 128 : (cch + 1) * 128],
                                ident[:],
                            )
                        for cch in range(CCH):
                            nc.vector.tensor_copy(
                                xT[:, cch, tc4 * 128 : tc4 * 128 + 128],
                                tr[:, cch * 128 : (cch + 1) * 128],
                            )
                    # ---- k, v (natural layout) + kv pair accumulation ----
                    for tc4 in range(4):
                        tsl = slice(tc4 * 128, tc4 * 128 + 128)
                        k_nat = kvpool.tile([128, C], BF16, tag="k")
                        v_nat = kvpool.tile([128, C], BF16, tag="v")
                        for js in range(3):  # j in [768+512js, 768+512(js+1))
                            pmm = ps_mm.tile([128, 512], F32, tag="pmm")
                            for cch in range(CCH):
                                nc.tensor.matmul(
                                    pmm[:],
                                    xT[:, cch, tsl],
                                    w_qkv[:, cch, C + js * 512 : C + js * 512 + 512],
                                    start=(cch == 0),
                                    stop=(cch == CCH - 1),
                                )
                            # k cols: js0 -> k[0:512], js1 -> k[512:768]+v[0:256]
                            if js == 0:
                                nc.vector.tensor_copy(k_nat[:, 0:512], pmm[:])
                            elif js == 1:
                                nc.vector.tensor_copy(k_nat[:, 512:768], pmm[:, 0:256])
                                nc.vector.tensor_copy(v_nat[:, 0:256], pmm[:, 256:512])
                            else:
                                nc.vector.tensor_copy(v_nat[:, 256:768], pmm[:])
                        first = ts == 0 and tc4 == 0
                        last = ts == NTS - 1 and tc4 == 3
                        for pr in range(NPAIR):
                            psl = slice(pr * 128, pr * 128 + 128)
                            # start=True clears has_written for the WHOLE psum
                            # bank, so only the first matmul touching each bank
                            # may set it (bank0: pairs 0-3, bank1: pairs 4-5).
                            nc.tensor.matmul(
                                kv_acc[:, psl],
                                k_nat[:, psl],
                                v_nat[:, psl],
                                start=(first and pr in (0, 4)),
                                stop=last,
                                skip_group_check=True,
                            )
                    # ---- q^T chunks with fused gelu ----
                    for jch in range(CCH):
                        pq = ps_q.tile([128, 512], F32, tag="pq")
                        for cch in range(CCH):
                            nc.tensor.matmul(
                                pq[:],
                                w_qkv[:, cch, jch * 128 : (jch + 1) * 128],
                                xT[:, cch, 0:512],
                                start=(cch == 0),
                                stop=(cch == CCH - 1),
                            )
                        nc.scalar.activation(
                            gqT[:, jch, ts * 512 : ts * 512 + 512], pq[:], GELU
                        )
                # ---- gelu(kv * scale) into per-head tiles ----
                for pr in range(NPAIR):
                    c0 = pr * 128
                    nc.scalar.activation(
                        gkv[0:64, pr, :],
                        kv_acc[0:64, c0 : c0 + 64],
                        GELU,
                        scale=SCALE,
                    )
                    nc.scalar.activation(
                        gkv[64:128, pr, :],
                        kv_acc[64:128, c0 + 64 : c0 + 128],
                        GELU,
                        scale=SCALE,
                    )

                # ================= pass 2: attention + projection ===========
                for ts in range(NTS):
                    tq = slice(ts * 512, ts * 512 + 512)
                    oT = otpool.tile([128, NPAIR, 512], BF16)
                    for pr in range(NPAIR):
                        po = ps_mm.tile([128, 512], F32, tag="pmm")
                        # two heads in disjoint 64-row/64-col sub-arrays;
                        # the two matmuls execute concurrently on the PE.
                        nc.tensor.matmul(
                            po[0:64, :], gkv[0:64, pr, :], gqT[0:64, pr, tq]
                        )
                        nc.tensor.matmul(
                            po[64:128, :], gkv[64:128, pr, :], gqT[64:128, pr, tq]
                        )
                        nc.vector.tensor_copy(oT[:, pr, :], po[:])
                    for tc4 in range(4):
                        tsl = slice(tc4 * 128, tc4 * 128 + 128)
                        py0 = ps_mm.tile([128, 512], F32, tag="pmm")
                        py1 = ps_mm.tile([128, 256], F32, tag="pmm")
                        for cch in range(CCH):
                            nc.tensor.matmul(
                                py0[:],
                                oT[:, cch, tsl],
                                w_proj[:, cch, 0:512],
                                start=(cch == 0),
                                stop=False,
                                skip_group_check=True,
                            )
                            nc.tensor.matmul(
                                py1[:],
                                oT[:, cch, tsl],
                                w_proj[:, cch, 512:768],
                                start=(cch == 0),
                                stop=False,
                                skip_group_check=True,
                            )
                        nc.tensor.matmul(
                            py0[:], ones1[:], b_row[:, 0:512],
                            start=False, stop=True, skip_group_check=True,
                        )
                        nc.tensor.matmul(
                            py1[:], ones1[:], b_row[:, 512:768],
                            start=False, stop=True, skip_group_check=True,
                        )
                        y_sb = ypool.tile([128, C], F32)
                        nc.scalar.activation(y_sb[:, 0:512], py0[:], COPY)
                        nc.scalar.activation(y_sb[:, 512:768], py1[:], COPY)
                        t0 = ts * 512 + tc4 * 128
                        nc.sync.dma_start(y_d[b, t0 : t0 + 128, :], y_sb[:])

    nc.compile()
    return nc


_cached_nc = None


def kernel(x, w_qkv, w_proj, b_proj):
    global _cached_nc
    if _cached_nc is None:
        _cached_nc = _build_program()
    nc = _cached_nc

    x = np.ascontiguousarray(x, dtype=np.float32)
    in_maps = [
        {
            "x": x[i * BPC : (i + 1) * BPC],
            "w_qkv": np.asarray(w_qkv, dtype=np.float32),
            "w_proj": np.asarray(w_proj, dtype=np.float32),
            "b_proj": np.asarray(b_proj, dtype=np.float32),
        }
        for i in range(NCORES)
    ]
    res = run_bass_kernel_spmd(nc, in_maps, core_ids=list(range(NCORES)))
    out = np.concatenate([res.results[i]["y"] for i in range(NCORES)], axis=0)
    return out.astype(np.float32)


# revision 8
# speedup vs baseline: 1.4754x; 1.1434x over previous
"""Trainium2 Bass kernel for AttentionSimple (linear/kernelized attention).

Computes, for x:[B,N,C], w_qkv:[C,3C], w_proj:[C,C], b_proj:[C]:
    qkv = x @ w_qkv -> split q,k,v per head (H=12, D=64)
    kv  = (k^T v) * D^-0.5          per (b, h)     [D, D]
    out = gelu(q) @ gelu(kv)        per (b, h)     [N, D]
    y   = out @ w_proj + b_proj

Sharding: data-parallel over batch B=16 across 8 NeuronCores (2 batches/core).
All matmuls run in bf16 with fp32 PSUM accumulation.

Dataflow per core (per batch b):
  pass 1: x tiles -> bf16 -> x^T via DMA-xbar transpose; k,v in natural
          [token, d] layout (lhsT = x^T chunk); per-head-pair k^T v
          accumulated in a persistent PSUM tile; q^T computed directly
          transposed (lhsT = w_q chunk, rhs = x^T) with gelu fused into the
          PSUM evacuation; gelu(kv * scale) into per-head [64,64] tiles.
  pass 2: attention o^T = gkv_h^T @ gq_h^T with two heads packed in
          disjoint 64-row/64-col PE sub-arrays (concurrent); projection
          consumes o^T directly (natural-layout output), bias folded in as
          a K=1 matmul row; contiguous DMA out.

Self-contained: hardcodes shapes; builds the Bass program, runs it SPMD on
cores 0-7 via bass_utils.run_bass_kernel_spmd, returns the gathered output.
"""

import numpy as np

import concourse.bacc as bacc
import concourse.bass as bass
import concourse.mybir as mybir
import concourse.tile as tile
from concourse import masks
from concourse.bass_utils import run_bass_kernel_spmd

F32 = mybir.dt.float32
BF16 = mybir.dt.bfloat16
GELU = mybir.ActivationFunctionType.Gelu
COPY = mybir.ActivationFunctionType.Copy
PSUM = bass.MemorySpace.PSUM

B, N, C = 16, 4096, 768
H, D = 12, 64
SCALE = D**-0.5
NCORES = 8
BPC = B // NCORES  # batches per core
CCH = C // 128  # 6 column chunks of 128
NTS = N // 512  # 8 slices of 512 tokens
NPAIR = H // 2  # 6 head pairs (128 cols each)


def _build_program():
    nc = bacc.Bacc("TRN2", target_bir_lowering=False, debug=False)

    x_d = nc.dram_tensor("x", [BPC, N, C], F32, kind="ExternalInput").ap()
    wq_d = nc.dram_tensor("w_qkv", [C, 3 * C], F32, kind="ExternalInput").ap()
    wp_d = nc.dram_tensor("w_proj", [C, C], F32, kind="ExternalInput").ap()
    bp_d = nc.dram_tensor("b_proj", [C], F32, kind="ExternalInput").ap()
    y_d = nc.dram_tensor("y", [BPC, N, C], F32, kind="ExternalOutput").ap()

    with tile.TileContext(nc) as tc:
        with (
            tc.tile_pool(name="weights", bufs=1) as wpool,
            tc.tile_pool(name="acts", bufs=1) as apool,
            tc.tile_pool(name="xin", bufs=3) as xpool,
            tc.tile_pool(name="xt", bufs=2) as xtpool,
            tc.tile_pool(name="kvnat", bufs=3) as kvpool,
            tc.tile_pool(name="ot", bufs=2) as otpool,
            tc.tile_pool(name="yout", bufs=3) as ypool,
            tc.tile_pool(name="ps_mm", bufs=4, space=PSUM) as ps_mm,
            tc.tile_pool(name="ps_kv", bufs=1, space=PSUM) as ps_kv,
            tc.tile_pool(name="ps_tr", bufs=2, space=PSUM) as ps_tr,
        ):
            # ---- weights / constants (once) ----
            w_qkv = wpool.tile([128, CCH, 3 * C], BF16)  # 27.6KB/part
            w_proj = wpool.tile([128, CCH, C], BF16)  # 9.2KB/part
            for cch in range(CCH):
                nc.gpsimd.dma_start(
                    w_qkv[:, cch, :], wq_d[cch * 128 : (cch + 1) * 128, :]
                )
                nc.gpsimd.dma_start(
                    w_proj[:, cch, :], wp_d[cch * 128 : (cch + 1) * 128, :]
                )
            b_row = wpool.tile([1, C], BF16)
            nc.gpsimd.dma_start(b_row[:], bp_d.unsqueeze(0))
            ones1 = wpool.tile([1, 128], BF16)
            nc.gpsimd.memset(ones1[:], 1.0)
            ident = wpool.tile([128, 128], BF16)
            masks.make_identity(nc, ident[:])

            for b in range(BPC):
                # gqT: q^T with gelu applied, [c=768, t=4096] as 6 chunks
                gqT = apool.tile([128, CCH, N], BF16, tag="gqT")
                # gkv: per-pair [64,64] gelu(kv*scale); even head on
                # partitions 0:64, odd head on partitions 64:128
                gkv = apool.tile([128, NPAIR, 64], BF16, tag="gkv")

                # ================= pass 1: qkv + kv accumulation ============
                kv_acc = ps_kv.tile([128, NPAIR * 128], F32)
                for ts in range(NTS):
                    xT = xtpool.tile([128, CCH, 512], BF16)
                    for tc4 in range(4):
                        t0 = ts * 512 + tc4 * 128
                        x_bf = xpool.tile([128, C], BF16)
                        nc.gpsimd.dma_start(x_bf[:], x_d[b, t0 : t0 + 128, :])
                        tr = ps_tr.tile([128, CCH * 128], BF16)
                        for cch in range(CCH):
                            nc.tensor.transpose(
                                tr[:, cch * 128 : (cch + 1) * 128],
                                x_bf[:, cch * 128 : (cch + 1) * 128],
                                ident[:],
                            )
                        for cch in range(CCH):
                            nc.vector.tensor_copy(
                                xT[:, cch, tc4 * 128 : tc4 * 128 + 128],
                                tr[:, cch * 128 : (cch + 1) * 128],
                            )
                    # ---- k, v (natural layout) + kv pair accumulation ----
                    for tc4 in range(4):
                        tsl = slice(tc4 * 128, tc4 * 128 + 128)
                        k_nat = kvpool.tile([128, C], BF16, tag="k")
                        v_nat = kvpool.tile([128, C], BF16, tag="v")
                        for js in range(3):  # j in [768+512js, 768+512(js+1))
                            pmm = ps_mm.tile([128, 512], F32, tag="pmm")
                            for cch in range(CCH):
                                nc.tensor.matmul(
                                    pmm[:],
                                    xT[:, cch, tsl],
                                    w_qkv[:, cch, C + js * 512 : C + js * 512 + 512],
                                    start=(cch == 0),
                                    stop=(cch == CCH - 1),
                                )
                            # k cols: js0 -> k[0:512], js1 -> k[512:768]+v[0:256]
                            if js == 0:
                                nc.vector.tensor_copy(k_nat[:, 0:512], pmm[:])
                            elif js == 1:
                                nc.vector.tensor_copy(k_nat[:, 512:768], pmm[:, 0:256])
                                nc.vector.tensor_copy(v_nat[:, 0:256], pmm[:, 256:512])
                            else:
                                nc.vector.tensor_copy(v_nat[:, 256:768], pmm[:])
                        first = ts == 0 and tc4 == 0
                        last = ts == NTS - 1 and tc4 == 3
                        for pr in range(NPAIR):
                            psl = slice(pr * 128, pr * 128 + 128)
                            # start=True clears has_written for the WHOLE psum
                            # bank, so only the first matmul touching each bank
                            # may set it (bank0: pairs 0-3, bank1: pairs 4-5).
                            nc.tensor.matmul(
                                kv_acc[:, psl],
                                k_nat[:, psl],
                                v_nat[:, psl],
                                start=(first and pr in (0, 4)),
                                stop=last,
                                skip_group_check=True,
                            )
                    # ---- q^T chunks with fused gelu ----
                    for jch in range(CCH):
                        pq = ps_mm.tile([128, 512], F32, tag="pmm")
                        for cch in range(CCH):
                            nc.tensor.matmul(
                                pq[:],
                                w_qkv[:, cch, jch * 128 : (jch + 1) * 128],
                                xT[:, cch, 0:512],
                                start=(cch == 0),
                                stop=(cch == CCH - 1),
                            )
                        nc.scalar.activation(
                            gqT[:, jch, ts * 512 : ts * 512 + 512], pq[:], GELU
                        )
                # ---- gelu(kv * scale) into per-head tiles ----
                for pr in range(NPAIR):
                    c0 = pr * 128
                    nc.scalar.activation(
                        gkv[0:64, pr, :],
                        kv_acc[0:64, c0 : c0 + 64],
                        GELU,
                        scale=SCALE,
                    )
                    nc.scalar.activation(
                        gkv[64:128, pr, :],
                        kv_acc[64:128, c0 + 64 : c0 + 128],
                        GELU,
                        scale=SCALE,
                    )

                # ================= pass 2: attention + projection ===========
                for ts in range(NTS):
                    tq = slice(ts * 512, ts * 512 + 512)
                    oT = otpool.tile([128, NPAIR, 512], BF16)
                    for pr in range(NPAIR):
                        po = ps_mm.tile([128, 512], F32, tag="pmm")
                        # two heads in disjoint 64-row/64-col sub-arrays;
                        # the two matmuls execute concurrently on the PE.
                        nc.tensor.matmul(
                            po[0:64, :], gkv[0:64, pr, :], gqT[0:64, pr, tq]
                        )
                        nc.tensor.matmul(
                            po[64:128, :], gkv[64:128, pr, :], gqT[64:128, pr, tq]
                        )
                        nc.vector.tensor_copy(oT[:, pr, :], po[:])
                    for tc4 in range(4):
                        tsl = slice(tc4 * 128, tc4 * 128 + 128)
                        py0 = ps_mm.tile([128, 512], F32, tag="pmm")
                        py1 = ps_mm.tile([128, 256], F32, tag="pmm")
                        for cch in range(CCH):
                            nc.tensor.matmul(
                                py0[:],
                                oT[:, cch, tsl],
                                w_proj[:, cch, 0:512],
                                start=(cch == 0),
                                stop=False,
                                skip_group_check=True,
                            )
                            nc.tensor.matmul(
                                py1[:],
                                oT[:, cch, tsl],
                                w_proj[:, cch, 512:768],
                                start=(cch == 0),
                                stop=False,
                                skip_group_check=True,
                            )
                        nc.tensor.matmul(
                            py0[:], ones1[:], b_row[:, 0:512],
                            start=False, stop=True, skip_group_check=True,
                        )
                        nc.tensor.matmul(
                            py1[:], ones1[:], b_row[:, 512:768],
                            start=False, stop=True, skip_group_check=True,
                        )
                        y_sb = ypool.tile([128, C], F32)
                        nc.scalar.activation(y_sb[:, 0:512], py0[:], COPY)
                        nc.scalar.activation(y_sb[:, 512:768], py1[:], COPY)
                        t0 = ts * 512 + tc4 * 128
                        nc.sync.dma_start(y_d[b, t0 : t0 + 128, :], y_sb[:])

    nc.compile()
    return nc


_cached_nc = None


def kernel(x, w_qkv, w_proj, b_proj):
    global _cached_nc
    if _cached_nc is None:
        _cached_nc = _build_program()
    nc = _cached_nc

    x = np.ascontiguousarray(x, dtype=np.float32)
    in_maps = [
        {
            "x": x[i * BPC : (i + 1) * BPC],
            "w_qkv": np.asarray(w_qkv, dtype=np.float32),
            "w_proj": np.asarray(w_proj, dtype=np.float32),
            "b_proj": np.asarray(b_proj, dtype=np.float32),
        }
        for i in range(NCORES)
    ]
    res = run_bass_kernel_spmd(nc, in_maps, core_ids=list(range(NCORES)))
    out = np.concatenate([res.results[i]["y"] for i in range(NCORES)], axis=0)
    return out.astype(np.float32)


# revision 10
# speedup vs baseline: 1.5410x; 1.0445x over previous
"""Trainium2 Bass kernel for AttentionSimple (linear/kernelized attention).

Computes, for x:[B,N,C], w_qkv:[C,3C], w_proj:[C,C], b_proj:[C]:
    qkv = x @ w_qkv -> split q,k,v per head (H=12, D=64)
    kv  = (k^T v) * D^-0.5          per (b, h)     [D, D]
    out = gelu(q) @ gelu(kv)        per (b, h)     [N, D]
    y   = out @ w_proj + b_proj

Sharding: data-parallel over batch B=16 across 8 NeuronCores (2 batches/core).
All matmuls run in bf16 with fp32 PSUM accumulation.

Dataflow per core (per batch b):
  pass 1: x tiles -> bf16 -> x^T via DMA-xbar transpose; k,v in natural
          [token, d] layout (lhsT = x^T chunk); per-head-pair k^T v
          accumulated in a persistent PSUM tile; q^T computed directly
          transposed (lhsT = w_q chunk, rhs = x^T) with gelu fused into the
          PSUM evacuation; gelu(kv * scale) into per-head [64,64] tiles.
  pass 2: attention o^T = gkv_h^T @ gq_h^T with two heads packed in
          disjoint 64-row/64-col PE sub-arrays (concurrent); projection
          consumes o^T directly (natural-layout output), bias folded in as
          a K=1 matmul row; contiguous DMA out.

Self-contained: hardcodes shapes; builds the Bass program, runs it SPMD on
cores 0-7 via bass_utils.run_bass_kernel_spmd, returns the gathered output.
"""

import numpy as np

import concourse.bacc as bacc
import concourse.bass as bass
import concourse.mybir as mybir
import concourse.tile as tile
from concourse import masks
from concourse.bass_utils import run_bass_kernel_spmd

F32 = mybir.dt.float32
BF16 = mybir.dt.bfloat16
GELU = mybir.ActivationFunctionType.Gelu
COPY = mybir.ActivationFunctionType.Copy
PSUM = bass.MemorySpace.PSUM

B, N, C = 16, 4096, 768
H, D = 12, 64
SCALE = D**-0.5
NCORES = 8
BPC = B // NCORES  # batches per core
CCH = C // 128  # 6 column chunks of 128
NTS = N // 512  # 8 slices of 512 tokens
NPAIR = H // 2  # 6 head pairs (128 cols each)


def _build_program():
    nc = bacc.Bacc("TRN2", target_bir_lowering=False, debug=False)

    x_d = nc.dram_tensor("x", [BPC, N, C], F32, kind="ExternalInput").ap()
    wq_d = nc.dram_tensor("w_qkv", [C, 3 * C], F32, kind="ExternalInput").ap()
    wp_d = nc.dram_tensor("w_proj", [C, C], F32, kind="ExternalInput").ap()
    bp_d = nc.dram_tensor("b_proj", [C], F32, kind="ExternalInput").ap()
    y_d = nc.dram_tensor("y", [BPC, N, C], F32, kind="ExternalOutput").ap()

    with tile.TileContext(nc) as tc:
        with (
            tc.tile_pool(name="weights", bufs=1) as wpool,
            tc.tile_pool(name="acts", bufs=1) as apool,
            tc.tile_pool(name="xin", bufs=6) as xpool,
            tc.tile_pool(name="xt", bufs=2) as xtpool,
            tc.tile_pool(name="kvnat", bufs=3) as kvpool,
            tc.tile_pool(name="ot", bufs=2) as otpool,
            tc.tile_pool(name="yout", bufs=3) as ypool,
            tc.tile_pool(name="ps_mm", bufs=4, space=PSUM) as ps_mm,
            tc.tile_pool(name="ps_kv", bufs=1, space=PSUM) as ps_kv,
            tc.tile_pool(name="ps_tr", bufs=2, space=PSUM) as ps_tr,
        ):
            # ---- constants first (cheap, unblock transposes) ----
            ones1 = wpool.tile([1, 128], BF16)
            nc.gpsimd.memset(ones1[:], 1.0)
            ident = wpool.tile([128, 128], BF16)
            masks.make_identity(nc, ident[:])

            # ---- prefetch the first token slice before the big weight DMAs
            # so the SWDGE rings deliver x(0,0) immediately and the PE can
            # start transposing while weights stream in.
            def load_x(b, ts):
                tiles = []
                for tc4 in range(4):
                    t0 = ts * 512 + tc4 * 128
                    x_bf = xpool.tile([128, C], BF16, tag="x_bf")
                    nc.gpsimd.dma_start(x_bf[:], x_d[b, t0 : t0 + 128, :])
                    tiles.append(x_bf)
                return tiles

            x_pre = load_x(0, 0)

            # ---- weights: k/v slices first (needed first), then q, proj ----
            w_qkv = wpool.tile([128, CCH, 3 * C], BF16)  # 27.6KB/part
            w_proj = wpool.tile([128, CCH, C], BF16)  # 9.2KB/part
            for cch in range(CCH):
                nc.gpsimd.dma_start(
                    w_qkv[:, cch, C:], wq_d[cch * 128 : (cch + 1) * 128, C:]
                )
            for cch in range(CCH):
                nc.gpsimd.dma_start(
                    w_qkv[:, cch, 0:C], wq_d[cch * 128 : (cch + 1) * 128, 0:C]
                )
            for cch in range(CCH):
                nc.gpsimd.dma_start(
                    w_proj[:, cch, :], wp_d[cch * 128 : (cch + 1) * 128, :]
                )
            b_row = wpool.tile([1, C], BF16)
            nc.gpsimd.dma_start(b_row[:], bp_d.unsqueeze(0))

            for b in range(BPC):
                # gqT: q^T with gelu applied, [c=768, t=4096] as 6 chunks
                gqT = apool.tile([128, CCH, N], BF16, tag="gqT")
                # gkv: per-pair [64,64] gelu(kv*scale); even head on
                # partitions 0:64, odd head on partitions 64:128
                gkv = apool.tile([128, NPAIR, 64], BF16, tag="gkv")

                # ================= pass 1: qkv + kv accumulation ============
                kv_acc = ps_kv.tile([128, NPAIR * 128], F32)
                for ts in range(NTS):
                    xT = xtpool.tile([128, CCH, 512], BF16)
                    x_tiles = x_pre if (b, ts) == (0, 0) else load_x(b, ts)
                    for tc4 in range(4):
                        x_bf = x_tiles[tc4]
                        tr = ps_tr.tile([128, CCH * 128], BF16)
                        for cch in range(CCH):
                            nc.tensor.transpose(
                                tr[:, cch * 128 : (cch + 1) * 128],
                                x_bf[:, cch * 128 : (cch + 1) * 128],
                                ident[:],
                            )
                        for cch in range(CCH):
                            nc.vector.tensor_copy(
                                xT[:, cch, tc4 * 128 : tc4 * 128 + 128],
                                tr[:, cch * 128 : (cch + 1) * 128],
                            )
                    # ---- k, v (natural layout) + kv pair accumulation ----
                    for tc4 in range(4):
                        tsl = slice(tc4 * 128, tc4 * 128 + 128)
                        k_nat = kvpool.tile([128, C], BF16, tag="k")
                        v_nat = kvpool.tile([128, C], BF16, tag="v")
                        for js in range(3):  # j in [768+512js, 768+512(js+1))
                            pmm = ps_mm.tile([128, 512], F32, tag="pmm")
                            for cch in range(CCH):
                                nc.tensor.matmul(
                                    pmm[:],
                                    xT[:, cch, tsl],
                                    w_qkv[:, cch, C + js * 512 : C + js * 512 + 512],
                                    start=(cch == 0),
                                    stop=(cch == CCH - 1),
                                )
                            # k cols: js0 -> k[0:512], js1 -> k[512:768]+v[0:256]
                            if js == 0:
                                nc.vector.tensor_copy(k_nat[:, 0:512], pmm[:])
                            elif js == 1:
                                nc.vector.tensor_copy(k_nat[:, 512:768], pmm[:, 0:256])
                                nc.vector.tensor_copy(v_nat[:, 0:256], pmm[:, 256:512])
                            else:
                                nc.vector.tensor_copy(v_nat[:, 256:768], pmm[:])
                        first = ts == 0 and tc4 == 0
                        last = ts == NTS - 1 and tc4 == 3
                        for pr in range(NPAIR):
                            psl = slice(pr * 128, pr * 128 + 128)
                            # start=True clears has_written for the WHOLE psum
                            # bank, so only the first matmul touching each bank
                            # may set it (bank0: pairs 0-3, bank1: pairs 4-5).
                            nc.tensor.matmul(
                                kv_acc[:, psl],
                                k_nat[:, psl],
                                v_nat[:, psl],
                                start=(first and pr in (0, 4)),
                                stop=last,
                                skip_group_check=True,
                            )
                    # ---- q^T chunks with fused gelu ----
                    for jch in range(CCH):
                        pq = ps_mm.tile([128, 512], F32, tag="pmm")
                        for cch in range(CCH):
                            nc.tensor.matmul(
                                pq[:],
                                w_qkv[:, cch, jch * 128 : (jch + 1) * 128],
                                xT[:, cch, 0:512],
                                start=(cch == 0),
                                stop=(cch == CCH - 1),
                            )
                        nc.scalar.activation(
                            gqT[:, jch, ts * 512 : ts * 512 + 512], pq[:], GELU
                        )
                # ---- gelu(kv * scale) into per-head tiles ----
                for pr in range(NPAIR):
                    c0 = pr * 128
                    nc.scalar.activation(
                        gkv[0:64, pr, :],
                        kv_acc[0:64, c0 : c0 + 64],
                        GELU,
                        scale=SCALE,
                    )
                    nc.scalar.activation(
                        gkv[64:128, pr, :],
                        kv_acc[64:128, c0 + 64 : c0 + 128],
                        GELU,
                        scale=SCALE,
                    )

                # ================= pass 2: attention + projection ===========
                for ts in range(NTS):
                    tq = slice(ts * 512, ts * 512 + 512)
                    oT = otpool.tile([128, NPAIR, 512], BF16)
                    for pr in range(NPAIR):
                        po = ps_mm.tile([128, 512], F32, tag="pmm")
                        # two heads in disjoint 64-row/64-col sub-arrays;
                        # the two matmuls execute concurrently on the PE.
                        nc.tensor.matmul(
                            po[0:64, :], gkv[0:64, pr, :], gqT[0:64, pr, tq]
                        )
                        nc.tensor.matmul(
                            po[64:128, :], gkv[64:128, pr, :], gqT[64:128, pr, tq]
                        )
                        nc.vector.tensor_copy(oT[:, pr, :], po[:])
                    for tc4 in range(4):
                        tsl = slice(tc4 * 128, tc4 * 128 + 128)
                        py0 = ps_mm.tile([128, 512], F32, tag="pmm")
                        py1 = ps_mm.tile([128, 256], F32, tag="pmm")
                        for cch in range(CCH):
                            nc.tensor.matmul(
                                py0[:],
                                oT[:, cch, tsl],
                                w_proj[:, cch, 0:512],
                                start=(cch == 0),
                                stop=False,
                                skip_group_check=True,
                            )
                            nc.tensor.matmul(
                                py1[:],
                                oT[:, cch, tsl],
                                w_proj[:, cch, 512:768],
                                start=(cch == 0),
                                stop=False,
                                skip_group_check=True,
                            )
                        nc.tensor.matmul(
                            py0[:], ones1[:], b_row[:, 0:512],
                            start=False, stop=True, skip_group_check=True,
                        )
                        nc.tensor.matmul(
                            py1[:], ones1[:], b_row[:, 512:768],
                            start=False, stop=True, skip_group_check=True,
                        )
                        y_sb = ypool.tile([128, C], F32)
                        nc.scalar.activation(y_sb[:, 0:512], py0[:], COPY)
                        nc.scalar.activation(y_sb[:, 512:768], py1[:], COPY)
                        t0 = ts * 512 + tc4 * 128
                        nc.sync.dma_start(y_d[b, t0 : t0 + 128, :], y_sb[:])

    nc.compile()
    return nc


_cached_nc = None


def kernel(x, w_qkv, w_proj, b_proj):
    global _cached_nc
    if _cached_nc is None:
        _cached_nc = _build_program()
    nc = _cached_nc

    x = np.ascontiguousarray(x, dtype=np.float32)
    in_maps = [
        {
            "x": x[i * BPC : (i + 1) * BPC],
            "w_qkv": np.asarray(w_qkv, dtype=np.float32),
            "w_proj": np.asarray(w_proj, dtype=np.float32),
            "b_proj": np.asarray(b_proj, dtype=np.float32),
        }
        for i in range(NCORES)
    ]
    res = run_bass_kernel_spmd(nc, in_maps, core_ids=list(range(NCORES)))
    out = np.concatenate([res.results[i]["y"] for i in range(NCORES)], axis=0)
    return out.astype(np.float32)
